# revision 12
# baseline (speedup 1.0000x reference)
"""Trainium2 Bass kernel for nn_CoeffToValue: separable cubic B-spline
coefficient-to-value filter ([1,4,1]/6 along each of D,H,W, zero padding).

Sharding: 8 cores = 4 (b,c) pairs x 2 H-halves of 96 rows.
Per-core layout: partitions = h (96+2 halo), free = (d, w).
  - W-filter: VectorE (shifted tensor_add + fused scalar_tensor_tensor)
  - H-filter: TensorE matmul contracting the h partition axis with a banded
    96x98 filter matrix (zero-padded halos make it identical on every core)
  - D-filter: fused into the same matmuls via 3 PSUM-accumulating taps with
    d-shifted rhs slices; stationaries F/216, 4F/216, F/216 carry the
    [1,4,1] d-taps and the global (1/6)^3 normalization
  - ScalarE evacuates PSUM -> SBUF, DMA writes h-major output slabs
"""

import sys

sys.path.insert(0, "/opt/trn_rl_repo")

import numpy as np

# Problem shape (hardcoded per harness contract)
B, C, D, H, W = 2, 2, 160, 192, 160
N_CORES = 8
H_SLAB = 96          # output h rows per core
H_IN = H_SLAB + 2    # input h rows incl. 1-row halo each side
D_TILE = 32          # output d rows per SBUF tile
N_DTILES = D // D_TILE
WP = W + 2           # zero-padded w extent
DP = D + 2           # zero-padded d extent
CHUNK = 512          # PSUM free-dim chunk (fp32 moving-operand max)
FLAT_OUT = D_TILE * W          # 5120, flat (d,w) out elems per tile
N_CHUNKS = FLAT_OUT // CHUNK   # 10

_PROGRAMS = {}


def _build_program(repeat=1):
    import concourse.mybir as mybir
    from concourse import bacc
    from concourse.bass import MemorySpace
    from concourse.tile import TileContext

    f32 = mybir.dt.float32
    nc = bacc.Bacc(None, target_bir_lowering=False, name="coeff_to_value")
    PSUM_BUFS = 4

    x = nc.dram_tensor("x", [H_IN, DP, WP], f32, kind="ExternalInput")
    fh1 = nc.dram_tensor("fh1", [H_IN, H_SLAB], f32, kind="ExternalInput")
    fh4 = nc.dram_tensor("fh4", [H_IN, H_SLAB], f32, kind="ExternalInput")
    y = nc.dram_tensor("y", [H_SLAB, D, W], f32, kind="ExternalOutput")

    with TileContext(nc) as tc:
        with (
            tc.tile_pool(name="consts", bufs=1) as cpool,
            tc.tile_pool(name="io", bufs=2) as iopool,
            tc.tile_pool(name="work", bufs=2) as wpool,
            tc.tile_pool(name="psum", bufs=PSUM_BUFS, space=MemorySpace.PSUM) as psum_pool,
        ):
            f1_t = cpool.tile([H_IN, H_SLAB], f32)
            f4_t = cpool.tile([H_IN, H_SLAB], f32)
            nc.sync.dma_start(out=f1_t[:], in_=fh1[:])
            nc.sync.dma_start(out=f4_t[:], in_=fh4[:])

            for t in [tt % N_DTILES for tt in range(repeat * N_DTILES)]:
                d0 = t * D_TILE
                # load [H_IN, D_TILE+2, WP] slab (input d rows d0-1..d0+32 in
                # padded coords = padded rows d0..d0+33)
                xt = iopool.tile([H_IN, D_TILE + 2, WP], f32, tag="xt")
                nc.sync.dma_start(out=xt[:], in_=x[:, d0 : d0 + D_TILE + 2, :])

                # W-filter (unnormalized [1,4,1]):
                #   u  = x[w-1] + x[w+1]
                #   x1 = 4*x[w] + u
                u = wpool.tile([H_IN, D_TILE + 2, W], f32, tag="u")
                nc.vector.tensor_add(
                    out=u[:],
                    in0=xt[:, :, 0:W],
                    in1=xt[:, :, 2 : W + 2],
                )
                x1 = wpool.tile([H_IN, D_TILE + 2, W], f32, tag="x1")
                nc.vector.scalar_tensor_tensor(
                    out=x1[:],
                    in0=xt[:, :, 1 : W + 1],
                    scalar=4.0,
                    in1=u[:],
                    op0=mybir.AluOpType.mult,
                    op1=mybir.AluOpType.add,
                )
                x1f = x1[:].rearrange("p a b -> p (a b)")

                yt = iopool.tile([H_SLAB, FLAT_OUT], f32, tag="yt")
                for k in range(N_CHUNKS):
                    ps = psum_pool.tile([H_SLAB, CHUNK], f32, tag="ps")
                    for j, stat in ((0, f1_t), (1, f4_t), (2, f1_t)):
                        off = k * CHUNK + j * W
                        nc.tensor.matmul(
                            ps[:],
                            stat[:],
                            x1f[:, off : off + CHUNK],
                            start=(j == 0),
                            stop=(j == 2),
                        )
                    nc.scalar.activation(
                        out=yt[:, k * CHUNK : (k + 1) * CHUNK],
                        in_=ps[:],
                        func=mybir.ActivationFunctionType.Copy,
                    )

                nc.scalar.dma_start(
                    out=y[:, d0 : d0 + D_TILE, :],
                    in_=yt[:].rearrange("p (a b) -> p a b", a=D_TILE, b=W),
                )

    nc.compile()
    return nc


def _get_program(repeat=1):
    if repeat not in _PROGRAMS:
        _PROGRAMS[repeat] = _build_program(repeat)
    return _PROGRAMS[repeat]


def _filter_matrices():
    # F[i, j] = tap weight of padded input h-row i on output h-row j.
    # Input rows 0..97 are global rows h0-1..h0+96 (row 0/97 zero-padded or
    # real halo); output j is global h0+j: taps at i = j, j+1, j+2.
    F = np.zeros((H_IN, H_SLAB), np.float32)
    jj = np.arange(H_SLAB)
    F[jj, jj] = 1.0
    F[jj + 1, jj] = 4.0
    F[jj + 2, jj] = 1.0
    return F * np.float32(1.0 / 216.0), F * np.float32(4.0 / 216.0)


def _make_in_maps(coeff):
    coeff = np.asarray(coeff, dtype=np.float32)
    pad = np.pad(coeff, ((0, 0), (0, 0), (1, 1), (1, 1), (1, 1)))
    fh1, fh4 = _filter_matrices()
    in_maps = []
    for i in range(N_CORES):
        b, c, h0 = (i // 2) // 2, (i // 2) % 2, (i % 2) * H_SLAB
        slab = pad[b, c, :, h0 : h0 + H_IN, :]  # [DP, H_IN, WP]
        xs = np.ascontiguousarray(slab.transpose(1, 0, 2))  # [H_IN, DP, WP]
        in_maps.append({"x": xs, "fh1": fh1, "fh4": fh4})
    return in_maps


def _gather(results):
    out = np.empty((B, C, D, H, W), np.float32)
    for i, r in enumerate(results):
        b, c, h0 = (i // 2) // 2, (i // 2) % 2, (i % 2) * H_SLAB
        out[b, c, :, h0 : h0 + H_SLAB, :] = r["y"].transpose(1, 0, 2)
    return out


def run(coeff, trace=False):
    from concourse.bass_utils import run_bass_kernel_spmd

    nc = _get_program()
    res = run_bass_kernel_spmd(
        nc, _make_in_maps(coeff), core_ids=list(range(N_CORES)), trace=trace
    )
    return _gather(res.results), res


def kernel(coeff):
    out, _ = run(coeff, trace=False)
    return out


def _pjrt_callable(nc):
    """Sharded jitted callable over 8 cores with NO donation, so inputs and
    zero-output seeds stay device-resident across repeated timing calls."""
    import jax
    import numpy as np
    from jax.sharding import Mesh, PartitionSpec
    from jax.experimental.shard_map import shard_map
    import concourse.mybir as mybir
    from concourse import bass2jax

    bass2jax.install_neuronx_cc_hook()
    in_names, out_names, out_avals, zero_outs = [], [], [], []
    partition_name = nc.partition_id_tensor.name if nc.partition_id_tensor else None
    for alloc in nc.m.functions[0].allocations:
        if not isinstance(alloc, mybir.MemoryLocationSet):
            continue
        name = alloc.memorylocations[0].name
        if alloc.kind == "ExternalInput":
            if name != partition_name:
                in_names.append(name)
        elif alloc.kind == "ExternalOutput":
            out_names.append(name)
            shape = tuple(alloc.tensor_shape)
            dtype = mybir.dt.np(alloc.dtype)
            out_avals.append(jax.core.ShapedArray(shape, dtype))
            zero_outs.append(np.zeros(shape, dtype))
    n_params = len(in_names)
    all_in_names = list(in_names) + list(out_names)
    if partition_name is not None:
        all_in_names.append(partition_name)

    def _body(*args):
        operands = list(args)
        if partition_name is not None:
            operands.append(bass2jax.partition_id_tensor())
        return tuple(
            bass2jax._bass_exec_p.bind(
                *operands,
                out_avals=tuple(out_avals),
                in_names=tuple(all_in_names),
                out_names=tuple(out_names),
                lowering_input_output_aliases=(),
                sim_require_finite=True,
                sim_require_nnan=True,
                nc=nc,
            )
        )

    devices = jax.devices()[:N_CORES]
    mesh = Mesh(np.array(devices), ("core",))
    specs = (PartitionSpec("core"),) * (n_params + len(out_names))
    fn = jax.jit(
        shard_map(
            _body,
            mesh=mesh,
            in_specs=specs,
            out_specs=(PartitionSpec("core"),) * len(out_names),
            check_rep=False,
        ),
        keep_unused=True,
    )
    return fn, in_names, out_names, zero_outs


def _device_args(in_maps, in_names, zero_outs):
    import numpy as np

    concat_in = [
        np.concatenate([np.asarray(m[name]) for m in in_maps], axis=0)
        for name in in_names
    ]
    concat_zeros = [
        np.zeros((N_CORES * z.shape[0], *z.shape[1:]), z.dtype) for z in zero_outs
    ]
    return concat_in + concat_zeros


def bench(coeff, repeat=8, iters=6):
    """Measure per-pipeline HW time via the wall-time slope between a
    repeat=R and a repeat=1 program with device-resident args."""
    import time
    import jax

    in_maps = _make_in_maps(coeff)
    results = {}
    for rep in (1, repeat):
        nc = _get_program(rep)
        fn, in_names, out_names, zero_outs = _pjrt_callable(nc)
        args = [jax.device_put(a) for a in _device_args(in_maps, in_names, zero_outs)]
        jax.block_until_ready(args)
        jax.block_until_ready(fn(*args))  # compile + warm
        ts = []
        for _ in range(iters):
            t0 = time.perf_counter()
            jax.block_until_ready(fn(*args))
            ts.append(time.perf_counter() - t0)
        results[rep] = min(ts)
    ns = (results[repeat] - results[1]) / (repeat - 1) * 1e9
    return ns, results


# revision 23
# speedup vs baseline: 1129.5155x; 1129.5155x over previous
"""Trainium2 Bass kernel for nn_CoeffToValue: separable cubic B-spline
coefficient-to-value filter ([1,4,1]/6 along each of D,H,W, zero padding).

Sharding: 8 cores = 4 (b,c) pairs x 2 H-halves of 96 rows.
Per-core layout: partitions = h (96+2 halo), free = (d, w).
  - W-filter on VectorE: shifted tensor_add + fused scalar_tensor_tensor.
  - H-filter on TensorE: banded 98x96 matrix contracting the h partition
    axis (host-zero-padded halos make the matrix identical on every core).
  - D-filter: split between VectorE pre-adds and extra PSUM-accumulated
    matmul taps, controlled by `taps` (1: fully elementwise, 2: pre-add
    only, 3: fully on TensorE).
  - ScalarE evacuates PSUM -> SBUF; DMA writes h-major output slabs.
"""

import sys

sys.path.insert(0, "/opt/trn_rl_repo")

import numpy as np

# Problem shape (hardcoded per harness contract)
B, C, D, H, W = 2, 2, 160, 192, 160
N_CORES = 8
H_SLAB = 96          # output h rows per core
H_IN = H_SLAB + 2    # input h rows incl. 1-row halo each side
WP = W + 2           # zero-padded w extent
DP = D + 2           # zero-padded d extent
CHUNK = 512          # PSUM free-dim chunk (fp32 moving-operand max)

_PROGRAMS = {}


def _build_program(repeat=1, d_tile=16, io_bufs=3, work_bufs=3, psum_bufs=4, taps=2):
    import concourse.mybir as mybir
    from concourse import bacc
    from concourse.bass import MemorySpace
    from concourse.tile import TileContext

    f32 = mybir.dt.float32
    add, mult = mybir.AluOpType.add, mybir.AluOpType.mult
    nc = bacc.Bacc(None, target_bir_lowering=False, name="coeff_to_value")
    DT = d_tile
    ND = D // DT
    FLAT = DT * W
    NCH = FLAT // CHUNK

    x = nc.dram_tensor("x", [H_IN, DP, WP], f32, kind="ExternalInput")
    fh1 = nc.dram_tensor("fh1", [H_IN, H_SLAB], f32, kind="ExternalInput")
    fh4 = nc.dram_tensor("fh4", [H_IN, H_SLAB], f32, kind="ExternalInput")
    y = nc.dram_tensor("y", [H_SLAB, D, W], f32, kind="ExternalOutput")

    with TileContext(nc) as tc:
        with (
            tc.tile_pool(name="consts", bufs=1) as cpool,
            tc.tile_pool(name="io", bufs=io_bufs) as iopool,
            tc.tile_pool(name="work", bufs=work_bufs) as wpool,
            tc.tile_pool(name="psum", bufs=psum_bufs, space=MemorySpace.PSUM) as psp,
        ):
            f1_t = cpool.tile([H_IN, H_SLAB], f32)
            nc.sync.dma_start(out=f1_t[:], in_=fh1[:])
            f4_t = None
            if taps >= 2:
                f4_t = cpool.tile([H_IN, H_SLAB], f32)
                nc.sync.dma_start(out=f4_t[:], in_=fh4[:])

            for t in [tt % ND for tt in range(repeat * ND)]:
                d0 = t * DT
                # load [H_IN, DT+2, WP]: padded-d rows d0..d0+DT+1
                xt = iopool.tile([H_IN, DT + 2, WP], f32, tag="xt")
                nc.sync.dma_start(out=xt[:], in_=x[:, d0 : d0 + DT + 2, :])

                # W-filter (unnormalized [1,4,1]):
                #   u = x[w-1] + x[w+1];  x1 = 4*x[w] + u   (in-place over u)
                u = wpool.tile([H_IN, DT + 2, W], f32, tag="u")
                nc.vector.tensor_add(
                    out=u[:], in0=xt[:, :, 0:W], in1=xt[:, :, 2 : W + 2]
                )
                nc.vector.scalar_tensor_tensor(
                    out=u[:], in0=xt[:, :, 1 : W + 1], scalar=4.0, in1=u[:],
                    op0=mult, op1=add,
                )
                uf = u[:].rearrange("p a b -> p (a b)")

                if taps == 1:
                    # D-filter fully elementwise: x2 = x1[d-1] + 4 x1[d] + x1[d+1]
                    x2 = wpool.tile([H_IN, DT, W], f32, tag="x2")
                    nc.vector.tensor_add(
                        out=x2[:], in0=u[:, 0:DT, :], in1=u[:, 2 : DT + 2, :]
                    )
                    nc.vector.scalar_tensor_tensor(
                        out=x2[:], in0=u[:, 1 : DT + 1, :], scalar=4.0, in1=x2[:],
                        op0=mult, op1=add,
                    )
                    x2f = x2[:].rearrange("p a b -> p (a b)")
                    mm_args = lambda k: [(f1_t, k * CHUNK, x2f)]
                elif taps == 2:
                    # D pre-add elementwise, center tap via 4F stationary
                    ad = wpool.tile([H_IN, DT, W], f32, tag="ad")
                    nc.vector.tensor_add(
                        out=ad[:], in0=u[:, 0:DT, :], in1=u[:, 2 : DT + 2, :]
                    )
                    adf = ad[:].rearrange("p a b -> p (a b)")
                    mm_args = lambda k: [
                        (f4_t, W + k * CHUNK, uf),
                        (f1_t, k * CHUNK, adf),
                    ]
                else:
                    # D-filter fully via 3 PSUM-accumulated taps
                    mm_args = lambda k: [
                        (f1_t, k * CHUNK, uf),
                        (f4_t, W + k * CHUNK, uf),
                        (f1_t, 2 * W + k * CHUNK, uf),
                    ]

                yt = iopool.tile([H_SLAB, FLAT], f32, tag="yt")
                for k in range(NCH):
                    ps = psp.tile([H_SLAB, CHUNK], f32, tag="ps")
                    args = mm_args(k)
                    for j, (stat, off, src) in enumerate(args):
                        nc.tensor.matmul(
                            ps[:], stat[:], src[:, off : off + CHUNK],
                            start=(j == 0), stop=(j == len(args) - 1),
                        )
                    nc.scalar.activation(
                        out=yt[:, k * CHUNK : (k + 1) * CHUNK], in_=ps[:],
                        func=mybir.ActivationFunctionType.Copy,
                    )

                nc.scalar.dma_start(
                    out=y[:, d0 : d0 + DT, :],
                    in_=yt[:].rearrange("p (a b) -> p a b", a=DT, b=W),
                )

    nc.compile()
    return nc


def _get_program(repeat=1, **kw):
    key = (repeat, tuple(sorted(kw.items())))
    if key not in _PROGRAMS:
        _PROGRAMS[key] = _build_program(repeat, **kw)
    return _PROGRAMS[key]


def _filter_matrices():
    # F[i, j] = tap weight of padded input h-row i on output h-row j.
    F = np.zeros((H_IN, H_SLAB), np.float32)
    jj = np.arange(H_SLAB)
    F[jj, jj] = 1.0
    F[jj + 1, jj] = 4.0
    F[jj + 2, jj] = 1.0
    return F * np.float32(1.0 / 216.0), F * np.float32(4.0 / 216.0)


def _make_in_maps(coeff):
    coeff = np.asarray(coeff, dtype=np.float32)
    pad = np.pad(coeff, ((0, 0), (0, 0), (1, 1), (1, 1), (1, 1)))
    fh1, fh4 = _filter_matrices()
    in_maps = []
    for i in range(N_CORES):
        b, c, h0 = (i // 2) // 2, (i // 2) % 2, (i % 2) * H_SLAB
        slab = pad[b, c, :, h0 : h0 + H_IN, :]  # [DP, H_IN, WP]
        xs = np.ascontiguousarray(slab.transpose(1, 0, 2))  # [H_IN, DP, WP]
        in_maps.append({"x": xs, "fh1": fh1, "fh4": fh4})
    return in_maps


def _gather(results):
    out = np.empty((B, C, D, H, W), np.float32)
    for i, r in enumerate(results):
        b, c, h0 = (i // 2) // 2, (i // 2) % 2, (i % 2) * H_SLAB
        out[b, c, :, h0 : h0 + H_SLAB, :] = r["y"].transpose(1, 0, 2)
    return out


def run(coeff, trace=False, **kw):
    from concourse.bass_utils import run_bass_kernel_spmd

    nc = _get_program(**kw)
    res = run_bass_kernel_spmd(
        nc, _make_in_maps(coeff), core_ids=list(range(N_CORES)), trace=trace
    )
    return _gather(res.results), res


def kernel(coeff):
    out, _ = run(coeff, trace=False)
    return out


def _pjrt_callable(nc):
    """Sharded jitted callable over 8 cores with NO donation, so inputs and
    zero-output seeds stay device-resident across repeated timing calls."""
    import jax
    from jax.sharding import Mesh, PartitionSpec
    from jax.experimental.shard_map import shard_map
    import concourse.mybir as mybir
    from concourse import bass2jax

    bass2jax.install_neuronx_cc_hook()
    in_names, out_names, out_avals, zero_outs = [], [], [], []
    partition_name = nc.partition_id_tensor.name if nc.partition_id_tensor else None
    for alloc in nc.m.functions[0].allocations:
        if not isinstance(alloc, mybir.MemoryLocationSet):
            continue
        name = alloc.memorylocations[0].name
        if alloc.kind == "ExternalInput":
            if name != partition_name:
                in_names.append(name)
        elif alloc.kind == "ExternalOutput":
            out_names.append(name)
            shape = tuple(alloc.tensor_shape)
            dtype = mybir.dt.np(alloc.dtype)
            out_avals.append(jax.core.ShapedArray(shape, dtype))
            zero_outs.append(np.zeros(shape, dtype))
    n_params = len(in_names)
    all_in_names = list(in_names) + list(out_names)
    if partition_name is not None:
        all_in_names.append(partition_name)

    def _body(*args):
        operands = list(args)
        if partition_name is not None:
            operands.append(bass2jax.partition_id_tensor())
        return tuple(
            bass2jax._bass_exec_p.bind(
                *operands,
                out_avals=tuple(out_avals),
                in_names=tuple(all_in_names),
                out_names=tuple(out_names),
                lowering_input_output_aliases=(),
                sim_require_finite=True,
                sim_require_nnan=True,
                nc=nc,
            )
        )

    devices = jax.devices()[:N_CORES]
    mesh = Mesh(np.array(devices), ("core",))
    specs = (PartitionSpec("core"),) * (n_params + len(out_names))
    fn = jax.jit(
        shard_map(
            _body, mesh=mesh, in_specs=specs,
            out_specs=(PartitionSpec("core"),) * len(out_names), check_rep=False,
        ),
        keep_unused=True,
    )
    return fn, in_names, out_names, zero_outs


def _device_args(in_maps, in_names, zero_outs):
    concat_in = [
        np.concatenate([np.asarray(m[name]) for m in in_maps], axis=0)
        for name in in_names
    ]
    concat_zeros = [
        np.zeros((N_CORES * z.shape[0], *z.shape[1:]), z.dtype) for z in zero_outs
    ]
    return concat_in + concat_zeros


def bench(coeff, repeat=16, iters=8, **kw):
    """Per-pipeline HW time from the wall-time slope between repeat=R and
    repeat=1 programs, alternating rounds with device-resident args."""
    import time
    import jax

    in_maps = _make_in_maps(coeff)
    fns = {}
    for rep in (1, repeat):
        nc = _get_program(rep, **kw)
        fn, in_names, out_names, zero_outs = _pjrt_callable(nc)
        args = [jax.device_put(a) for a in _device_args(in_maps, in_names, zero_outs)]
        jax.block_until_ready(args)
        for _ in range(4):  # deep warmup: first calls after compile run slow
            jax.block_until_ready(fn(*args))
        fns[rep] = (fn, args)
    samples = {1: [], repeat: []}
    for _ in range(iters):
        for rep in (1, repeat):
            fn, args = fns[rep]
            t0 = time.perf_counter()
            jax.block_until_ready(fn(*args))
            samples[rep].append(time.perf_counter() - t0)
    ns = (min(samples[repeat]) - min(samples[1])) / (repeat - 1) * 1e9
    return ns, {k: min(v) for k, v in samples.items()}


# revision 27
# speedup vs baseline: 1357.2777x; 1.2016x over previous
"""Trainium2 Bass kernel for nn_CoeffToValue: separable cubic B-spline
coefficient-to-value filter ([1,4,1]/6 along each of D,H,W, zero padding).

Sharding: 8 cores = 4 (b,c) pairs x 2 H-halves of 96 rows.
Per-core layout: partitions = h (96+2 halo), free = (d, w).
  - W-filter on VectorE: shifted tensor_add + fused scalar_tensor_tensor.
  - H-filter on TensorE: banded 98x96 matrix contracting the h partition
    axis (host-zero-padded halos make the matrix identical on every core).
  - D-filter: split between VectorE pre-adds and extra PSUM-accumulated
    matmul taps, controlled by `taps` (1: fully elementwise, 2: pre-add
    only, 3: fully on TensorE).
  - ScalarE evacuates PSUM -> SBUF; DMA writes h-major output slabs.
"""

import sys

sys.path.insert(0, "/opt/trn_rl_repo")

import numpy as np

# Problem shape (hardcoded per harness contract)
B, C, D, H, W = 2, 2, 160, 192, 160
N_CORES = 8
H_SLAB = 96          # output h rows per core
H_IN = H_SLAB + 2    # input h rows incl. 1-row halo each side
WP = W + 2           # zero-padded w extent
DP = D + 2           # zero-padded d extent
CHUNK = 512          # PSUM free-dim chunk (fp32 moving-operand max)

_PROGRAMS = {}


def _build_program(repeat=1, d_tile=16, io_bufs=3, work_bufs=3, psum_bufs=6, taps=2, use_gp=False):
    import concourse.mybir as mybir
    from concourse import bacc
    from concourse.bass import MemorySpace
    from concourse.tile import TileContext

    f32 = mybir.dt.float32
    add, mult = mybir.AluOpType.add, mybir.AluOpType.mult
    nc = bacc.Bacc(None, target_bir_lowering=False, name="coeff_to_value")
    DT = d_tile
    ND = D // DT
    FLAT = DT * W
    NCH = FLAT // CHUNK

    x = nc.dram_tensor("x", [H_IN, DP, WP], f32, kind="ExternalInput")
    fh1 = nc.dram_tensor("fh1", [H_IN, H_SLAB], f32, kind="ExternalInput")
    fh4 = nc.dram_tensor("fh4", [H_IN, H_SLAB], f32, kind="ExternalInput")
    y = nc.dram_tensor("y", [H_SLAB, D, W], f32, kind="ExternalOutput")

    with TileContext(nc) as tc:
        with (
            tc.tile_pool(name="consts", bufs=1) as cpool,
            tc.tile_pool(name="io", bufs=io_bufs) as iopool,
            tc.tile_pool(name="work", bufs=work_bufs) as wpool,
            tc.tile_pool(name="psum", bufs=psum_bufs, space=MemorySpace.PSUM) as psp,
        ):
            f1_t = cpool.tile([H_IN, H_SLAB], f32)
            nc.sync.dma_start(out=f1_t[:], in_=fh1[:])
            f4_t = None
            if taps >= 2:
                f4_t = cpool.tile([H_IN, H_SLAB], f32)
                nc.sync.dma_start(out=f4_t[:], in_=fh4[:])

            for t in [tt % ND for tt in range(repeat * ND)]:
                d0 = t * DT
                # load [H_IN, DT+2, WP]: padded-d rows d0..d0+DT+1
                xt = iopool.tile([H_IN, DT + 2, WP], f32, tag="xt")
                nc.sync.dma_start(out=xt[:], in_=x[:, d0 : d0 + DT + 2, :])

                # W-filter (unnormalized [1,4,1]):
                #   u = x[w-1] + x[w+1];  x1 = 4*x[w] + u
                # With use_gp, GPSIMD takes ~47% of the plain adds (walrus
                # can't lower fused STT on Pool) and nothing runs in place.
                u = wpool.tile([H_IN, DT + 2, W], f32, tag="u")
                rw = round((DT + 2) * 0.47) if use_gp else DT + 2
                for eng, rs in ((nc.vector, slice(0, rw)), (nc.gpsimd, slice(rw, DT + 2))):
                    if rs.start >= rs.stop:
                        continue
                    eng.tensor_add(
                        out=u[:, rs, :], in0=xt[:, rs, 0:W], in1=xt[:, rs, 2 : W + 2]
                    )
                if use_gp:
                    x1 = wpool.tile([H_IN, DT + 2, W], f32, tag="x1")
                else:
                    x1 = u
                nc.vector.scalar_tensor_tensor(
                    out=x1[:], in0=xt[:, :, 1 : W + 1], scalar=4.0, in1=u[:],
                    op0=mult, op1=add,
                )
                u = x1
                uf = u[:].rearrange("p a b -> p (a b)")

                if taps == 1:
                    # D-filter fully elementwise: x2 = x1[d-1] + 4 x1[d] + x1[d+1]
                    x2 = wpool.tile([H_IN, DT, W], f32, tag="x2")
                    nc.vector.tensor_add(
                        out=x2[:], in0=u[:, 0:DT, :], in1=u[:, 2 : DT + 2, :]
                    )
                    nc.vector.scalar_tensor_tensor(
                        out=x2[:], in0=u[:, 1 : DT + 1, :], scalar=4.0, in1=x2[:],
                        op0=mult, op1=add,
                    )
                    x2f = x2[:].rearrange("p a b -> p (a b)")
                    mm_args = lambda k: [(f1_t, k * CHUNK, x2f)]
                elif taps == 2:
                    # D pre-add elementwise, center tap via 4F stationary
                    ad = wpool.tile([H_IN, DT, W], f32, tag="ad")
                    rd = round(DT * 0.47) if use_gp else DT
                    for eng, rs, rsm in (
                        (nc.vector, slice(0, rd), slice(2, rd + 2)),
                        (nc.gpsimd, slice(rd, DT), slice(rd + 2, DT + 2)),
                    ):
                        if rs.start >= rs.stop:
                            continue
                        eng.tensor_add(
                            out=ad[:, rs, :], in0=u[:, rs, :], in1=u[:, rsm, :]
                        )
                    adf = ad[:].rearrange("p a b -> p (a b)")
                    mm_args = lambda k: [
                        (f4_t, W + k * CHUNK, uf),
                        (f1_t, k * CHUNK, adf),
                    ]
                else:
                    # D-filter fully via 3 PSUM-accumulated taps
                    mm_args = lambda k: [
                        (f1_t, k * CHUNK, uf),
                        (f4_t, W + k * CHUNK, uf),
                        (f1_t, 2 * W + k * CHUNK, uf),
                    ]

                yt = iopool.tile([H_SLAB, FLAT], f32, tag="yt")
                if taps == 2:
                    # Group matmuls by stationary so the fp32 self-loading
                    # weight swap happens 2x per tile instead of 2x per chunk,
                    # and the 4F pass starts before the D pre-add finishes.
                    pss = [
                        psp.tile([H_SLAB, CHUNK], f32, tag="ps", name=f"ps{k}")
                        for k in range(NCH)
                    ]
                    for k in range(NCH):
                        off = W + k * CHUNK
                        nc.tensor.matmul(
                            pss[k][:], f4_t[:], uf[:, off : off + CHUNK],
                            start=True, stop=False,
                        )
                    for k in range(NCH):
                        off = k * CHUNK
                        nc.tensor.matmul(
                            pss[k][:], f1_t[:], adf[:, off : off + CHUNK],
                            start=False, stop=True,
                        )
                        nc.scalar.activation(
                            out=yt[:, k * CHUNK : (k + 1) * CHUNK], in_=pss[k][:],
                            func=mybir.ActivationFunctionType.Copy,
                        )
                else:
                    for k in range(NCH):
                        ps = psp.tile([H_SLAB, CHUNK], f32, tag="ps")
                        args = mm_args(k)
                        for j, (stat, off, src) in enumerate(args):
                            nc.tensor.matmul(
                                ps[:], stat[:], src[:, off : off + CHUNK],
                                start=(j == 0), stop=(j == len(args) - 1),
                            )
                        nc.scalar.activation(
                            out=yt[:, k * CHUNK : (k + 1) * CHUNK], in_=ps[:],
                            func=mybir.ActivationFunctionType.Copy,
                        )

                nc.scalar.dma_start(
                    out=y[:, d0 : d0 + DT, :],
                    in_=yt[:].rearrange("p (a b) -> p a b", a=DT, b=W),
                )

    nc.compile()
    return nc


def _get_program(repeat=1, **kw):
    key = (repeat, tuple(sorted(kw.items())))
    if key not in _PROGRAMS:
        _PROGRAMS[key] = _build_program(repeat, **kw)
    return _PROGRAMS[key]


def _filter_matrices():
    # F[i, j] = tap weight of padded input h-row i on output h-row j.
    F = np.zeros((H_IN, H_SLAB), np.float32)
    jj = np.arange(H_SLAB)
    F[jj, jj] = 1.0
    F[jj + 1, jj] = 4.0
    F[jj + 2, jj] = 1.0
    return F * np.float32(1.0 / 216.0), F * np.float32(4.0 / 216.0)


def _make_in_maps(coeff):
    coeff = np.asarray(coeff, dtype=np.float32)
    pad = np.pad(coeff, ((0, 0), (0, 0), (1, 1), (1, 1), (1, 1)))
    fh1, fh4 = _filter_matrices()
    in_maps = []
    for i in range(N_CORES):
        b, c, h0 = (i // 2) // 2, (i // 2) % 2, (i % 2) * H_SLAB
        slab = pad[b, c, :, h0 : h0 + H_IN, :]  # [DP, H_IN, WP]
        xs = np.ascontiguousarray(slab.transpose(1, 0, 2))  # [H_IN, DP, WP]
        in_maps.append({"x": xs, "fh1": fh1, "fh4": fh4})
    return in_maps


def _gather(results):
    out = np.empty((B, C, D, H, W), np.float32)
    for i, r in enumerate(results):
        b, c, h0 = (i // 2) // 2, (i // 2) % 2, (i % 2) * H_SLAB
        out[b, c, :, h0 : h0 + H_SLAB, :] = r["y"].transpose(1, 0, 2)
    return out


def run(coeff, trace=False, **kw):
    from concourse.bass_utils import run_bass_kernel_spmd

    nc = _get_program(**kw)
    res = run_bass_kernel_spmd(
        nc, _make_in_maps(coeff), core_ids=list(range(N_CORES)), trace=trace
    )
    return _gather(res.results), res


def kernel(coeff):
    out, _ = run(coeff, trace=False)
    return out


def _pjrt_callable(nc):
    """Sharded jitted callable over 8 cores with NO donation, so inputs and
    zero-output seeds stay device-resident across repeated timing calls."""
    import jax
    from jax.sharding import Mesh, PartitionSpec
    from jax.experimental.shard_map import shard_map
    import concourse.mybir as mybir
    from concourse import bass2jax

    bass2jax.install_neuronx_cc_hook()
    in_names, out_names, out_avals, zero_outs = [], [], [], []
    partition_name = nc.partition_id_tensor.name if nc.partition_id_tensor else None
    for alloc in nc.m.functions[0].allocations:
        if not isinstance(alloc, mybir.MemoryLocationSet):
            continue
        name = alloc.memorylocations[0].name
        if alloc.kind == "ExternalInput":
            if name != partition_name:
                in_names.append(name)
        elif alloc.kind == "ExternalOutput":
            out_names.append(name)
            shape = tuple(alloc.tensor_shape)
            dtype = mybir.dt.np(alloc.dtype)
            out_avals.append(jax.core.ShapedArray(shape, dtype))
            zero_outs.append(np.zeros(shape, dtype))
    n_params = len(in_names)
    all_in_names = list(in_names) + list(out_names)
    if partition_name is not None:
        all_in_names.append(partition_name)

    def _body(*args):
        operands = list(args)
        if partition_name is not None:
            operands.append(bass2jax.partition_id_tensor())
        return tuple(
            bass2jax._bass_exec_p.bind(
                *operands,
                out_avals=tuple(out_avals),
                in_names=tuple(all_in_names),
                out_names=tuple(out_names),
                lowering_input_output_aliases=(),
                sim_require_finite=True,
                sim_require_nnan=True,
                nc=nc,
            )
        )

    devices = jax.devices()[:N_CORES]
    mesh = Mesh(np.array(devices), ("core",))
    specs = (PartitionSpec("core"),) * (n_params + len(out_names))
    fn = jax.jit(
        shard_map(
            _body, mesh=mesh, in_specs=specs,
            out_specs=(PartitionSpec("core"),) * len(out_names), check_rep=False,
        ),
        keep_unused=True,
    )
    return fn, in_names, out_names, zero_outs


def _device_args(in_maps, in_names, zero_outs):
    concat_in = [
        np.concatenate([np.asarray(m[name]) for m in in_maps], axis=0)
        for name in in_names
    ]
    concat_zeros = [
        np.zeros((N_CORES * z.shape[0], *z.shape[1:]), z.dtype) for z in zero_outs
    ]
    return concat_in + concat_zeros


def bench(coeff, repeat=16, iters=8, **kw):
    """Per-pipeline HW time from the wall-time slope between repeat=R and
    repeat=1 programs, alternating rounds with device-resident args."""
    import time
    import jax

    in_maps = _make_in_maps(coeff)
    fns = {}
    for rep in (1, repeat):
        nc = _get_program(rep, **kw)
        fn, in_names, out_names, zero_outs = _pjrt_callable(nc)
        args = [jax.device_put(a) for a in _device_args(in_maps, in_names, zero_outs)]
        jax.block_until_ready(args)
        for _ in range(4):  # deep warmup: first calls after compile run slow
            jax.block_until_ready(fn(*args))
        fns[rep] = (fn, args)
    samples = {1: [], repeat: []}
    for _ in range(iters):
        for rep in (1, repeat):
            fn, args = fns[rep]
            t0 = time.perf_counter()
            jax.block_until_ready(fn(*args))
            samples[rep].append(time.perf_counter() - t0)
    ns = (min(samples[repeat]) - min(samples[1])) / (repeat - 1) * 1e9
    return ns, {k: min(v) for k, v in samples.items()}
